# revision 68
# baseline (speedup 1.0000x reference)
"""Back-projection kernel for Trainium2 (8 NeuronCores) — adaptive regions.

See kernel.py docstring for the math.  This version additionally lets every
angle choose its own w-region width from {8,16,32,64,128} (wide regions for
axis-aligned angles whose u-band barely depends on w; narrow regions for
diagonal angles), minimizing streamed T+p bytes.  Bands are continuously
packed per (class, region) and may split across bins.
"""

import numpy as np
import ml_dtypes

B, NANG, L = 2, 96, 128
NA2 = NANG // 2
H = W = U = L
NCORES = 8
HPC = H // NCORES       # 16 output rows per core
BZ = B * L              # 256 (b,z) columns
PAIR = 2 * BZ           # 512 = [p_a | p_{a+48}] column block
DELTA = 1e-11
NCH = 16                # px chunks (128 px each: 16h x 8w)
CW = 8
WSOPTS = (8, 16, 32, 64, 128)
PDMA_COLS = 3072        # merge p blocks into DMAs of >= ~0.75MB
_PE_GATE_CAP = 4        # max chunks of PE start delay (tunable)

_cache = {}


def _host_maps(angles: np.ndarray):
    a = angles.astype(np.float32)
    phi = (np.float32(270.0) - a).astype(np.float32)
    th = (phi * np.float32(np.pi / 180.0)).astype(np.float32)
    c = np.cos(th).astype(np.float32)[:, None, None]
    s = np.sin(th).astype(np.float32)[:, None, None]
    cy = cx = np.float32((L - 1) / 2.0)
    hh, ww = np.meshgrid(np.arange(H, dtype=np.float32),
                         np.arange(W, dtype=np.float32), indexing="ij")
    xr = (ww - cx)[None]
    yr = (hh - cy)[None]
    sx = (c * xr + s * yr + cx).astype(np.float32)
    sy = (-s * xr + c * yr + cy).astype(np.float32)
    x0 = np.floor(sx)
    y0 = np.floor(sy)
    fx = (sx - x0).astype(np.float64)
    fy = (sy - y0).astype(np.float64)
    x0i = x0.astype(np.int64)
    y0i = y0.astype(np.int64)
    my0 = ((y0i >= 0) & (y0i < H)).astype(np.float64)
    my1 = ((y0i + 1 >= 0) & (y0i + 1 < H)).astype(np.float64)
    mx0 = ((x0i >= 0) & (x0i < W)).astype(np.float64)
    mx1 = ((x0i + 1 >= 0) & (x0i + 1 < W)).astype(np.float64)
    wyv = (1.0 - fy) * my0 + fy * my1
    W0 = wyv * (1.0 - fx) * mx0
    W1 = wyv * fx * mx1
    I0 = np.clip(x0i, 0, W - 1)
    I1 = np.clip(x0i + 1, 0, W - 1)
    return W0, W1, I0, I1


def _make_plan(angles: np.ndarray):
    W0, W1, I0, I1 = _host_maps(angles)
    norm = (W0 + W1).sum(axis=0)
    inv = (1.0 / (norm + DELTA))[None]

    T = np.zeros((NA2, U, H, W), dtype=np.float32)
    ai = np.arange(NA2)[:, None, None]
    hi = np.arange(H)[None, :, None]
    wi = np.arange(W)[None, None, :]
    sh = I0[:NA2].shape
    aib = np.broadcast_to(ai, sh)
    hib = np.broadcast_to(hi, sh)
    wib = np.broadcast_to(wi, sh)
    T[aib, I0[:NA2], hib, wib] += (W0[:NA2] * inv).astype(np.float32)
    T[aib, I1[:NA2], hib, wib] += (W1[:NA2] * inv).astype(np.float32)

    big = 999
    I0m = np.where(W0 > 0, I0, big)[:NA2]
    I1m = np.where(W1 > 0, I1, big)[:NA2]
    I0M = np.where(W0 > 0, I0, -1)[:NA2]
    I1M = np.where(W1 > 0, I1, -1)[:NA2]

    lo_ws, wd_ws = {}, {}
    for ws in WSOPTS:
        R = W // ws
        lo = np.minimum(I0m, I1m).reshape(NA2, NCORES, HPC, R, ws).min(axis=(2, 4))
        hi_ = np.maximum(I0M, I1M).reshape(NA2, NCORES, HPC, R, ws).max(axis=(2, 4))
        wd = np.where(hi_ >= 0, hi_ - np.where(lo == big, 0, lo) + 1, 0)
        lo_ws[ws] = lo                     # [a, core, R]
        wd_ws[ws] = wd.max(axis=1)         # [a, R]

    # class-subset + per-angle assignment: minimize max(DMA time, PE time).
    # bin-tail padding (ceil to 128 rows per class-region) is modeled, and a
    # local search moves angles between classes to fill bins.
    def plan_cost(subset):
        chosen_ = []
        for a in range(NA2):
            best = None
            for ws in subset:
                rows = int(wd_ws[ws][a].sum())
                cost = rows * (HPC * ws + PAIR)
                if best is None or cost < best[0]:
                    best = (cost, ws)
            chosen_.append(best[1])

        def totals(ch):
            tcols_ = pcols_ = 0
            for ws in subset:
                R = W // ws
                for r in range(R):
                    rows = sum(int(wd_ws[ws][a, r]) for a in range(NA2)
                               if ch[a] == ws)
                    nbv = -(-rows // 128)
                    tcols_ += nbv * HPC * ws
                    pcols_ += nbv * PAIR
            return tcols_, pcols_

        tc, pc = totals(chosen_)
        improved = True
        sweeps = 0
        while improved and sweeps < 4:
            improved = False
            sweeps += 1
            for a in range(NA2):
                cur = chosen_[a]
                best = (tc + pc * 1.0, cur, tc, pc)
                for ws in subset:
                    if ws == cur:
                        continue
                    chosen_[a] = ws
                    tc2, pc2 = totals(chosen_)
                    if tc2 + pc2 < best[0]:
                        best = (tc2 + pc2, ws, tc2, pc2)
                chosen_[a] = best[1]
                if best[1] != cur:
                    improved = True
                    tc, pc = best[2], best[3]
        return chosen_, tc, pc

    best_sub = None
    # ws=128 has a single (self-mirrored) region and cannot use the w-flip
    # trick, so it is excluded
    for subset in ((16, 32), (16, 32, 64), (16, 64), (32,), (16,)):
        chosen_, tc, pc = plan_cost(subset)
        # calibrated on traced runs: DMA path = init + transfer + slack
        # (only half of T is streamed thanks to the w-flip mirror);
        # PE path = start latency + cycles (mild ramp factor) + drain tail
        dma_ns = 2330 + (tc / 2 + pc + NCH * PAIR) * 128 * 2 / 360.0 + 1500
        pe_ns = 6500 + tc / 128 * 512 / 2.4 * 1.05 + 1200 + 1800
        score = max(dma_ns, pe_ns)
        if best_sub is None or score < best_sub[0]:
            best_sub = (score, subset, chosen_)
    _, subset, chosen = best_sub
    # ascending: narrow classes first, so chunk 0's first matmuls need only
    # the small first p blocks and the PE starts sooner
    classes = sorted(set(subset))

    # w-flip symmetry: for pairs q>=25 use the OTHER 180-degree
    # representative T_{q+48} == wflip(T_{48-q}), so the right-half regions'
    # T blocks are exact wl-reversed copies of the left half and only
    # chunks 0-7 of T are streamed from HBM (the DVE mirrors the rest).
    # Mirror-pair class assignments and band extents accordingly.
    for x in range(25, NA2):
        chosen[x] = chosen[NA2 - x]
    wd_rep, lo_rep = {}, {}
    for ws in WSOPTS:
        wdr = wd_ws[ws].copy()
        lor = lo_ws[ws].copy()
        wdr[25:] = wd_ws[ws][23:0:-1, ::-1]
        lor[25:] = lo_ws[ws][23:0:-1, :, ::-1]
        wd_rep[ws] = wdr
        lo_rep[ws] = lor

    # continuous packing per (class, left region); right regions mirror the
    # left layout exactly (pair x at the same offset as 48-x)
    items = {}   # (ci, r) -> list of (pair, gpos, w)
    nb = {}      # (ci, r) -> bins
    for ci, ws in enumerate(classes):
        R = W // ws
        for r in range(R // 2):
            pos = 0
            its = []
            for a in range(NA2):
                if chosen[a] != ws:
                    continue
                w_ = int(wd_rep[ws][a, r])
                if w_ <= 0:
                    continue
                its.append((a, pos, w_))
                pos += w_
            items[(ci, r)] = its
            nb[(ci, r)] = -(-pos // 128)
            rm = R - 1 - r
            items[(ci, rm)] = [((NA2 - a) % NA2, off, w_)
                               for a, off, w_ in its]
            nb[(ci, rm)] = nb[(ci, r)]

    # unified load stream: T blocks (chunk-major, ascending class) and p
    # blocks (first-use order) interleaved exactly in PE consumption order,
    # then merged into DMA pieces.  Every piece gets its own semaphore and
    # every matmul records the piece index it depends on, so the PE's waits
    # are as fine as possible and the SP stream itself never waits.
    # pair 24 is the one self-mirror pair whose w-flip yields the OTHER
    # 180-degree member, leaving "angle 24 at right pixels" and "angle 72 at
    # left pixels" uncovered by the mirror.  Its mirrored p rows are zeroed
    # and small "patch" blocks (true T_24 at right-region columns) are
    # streamed for the right chunks; their acc1/acc2 provide exactly the
    # missing combinations.
    ci24 = classes.index(chosen[24])
    ws24 = classes[ci24]
    nlc24 = ws24 // CW
    wd24 = {r: int(wd_ws[ws24][24, r]) for r in range(W // ws24)}

    p_off = {}
    tbase = {}
    t_off = [0] * (NCH + 1)
    stream = []   # (src, lo, hi, first_use_chunk) block granules in order
    patch_p = {}
    patch_q = {}
    pcol = 0
    tco = 0
    qpos = 0
    for c in range(NCH):
        t_off[c] = tco
        for ci, ws in enumerate(classes):
            r = (CW * c) // ws
            tblk = nb[(ci, r)] * 128
            if c < NCH // 2:
                # right-half chunks' T is DVE-mirrored on chip, not streamed.
                # T goes BEFORE the (larger) p block so the PE's first matmul
                # is not queued behind a whole region of p.
                stream.append(("t", tco, tco + tblk, c))
            for g in range(nb[(ci, r)]):
                tbase[(c, ci, g)] = tco + g * 128
            tco += tblk
            if r * ws == CW * c:
                p_off[(ci, r)] = pcol
                blk = nb[(ci, r)] * PAIR
                if c < 3:
                    # per-bin granules so the merge can cut fine pieces and
                    # the PE's first matmuls wait on minimal data
                    for g in range(nb[(ci, r)]):
                        stream.append(("p", pcol + g * PAIR,
                                       pcol + (g + 1) * PAIR, c))
                else:
                    stream.append(("p", pcol, pcol + blk, c))
                pcol += blk
        if c >= NCH // 2:
            r24 = (CW * c) // ws24
            if r24 * ws24 == CW * c:
                patch_p[r24] = pcol
                stream.append(("p", pcol, pcol + PAIR, c))
                pcol += PAIR
                patch_q[r24] = qpos
                stream.append(("q", qpos, qpos + nlc24 * 128, c))
                qpos += nlc24 * 128
    t_off[NCH] = tco
    pcols = pcol
    qcols = qpos
    # give the T stream a 2-chunk lead over p: chunk 7's T is the source of
    # the first-needed mirror copy (chunk 8), so late T stalls the PE twice
    stream = sorted(
        stream,
        key=lambda e: (max(e[3] - 1, 0) if e[0] == "t" else e[3]),
    )

    # merge adjacent same-src granules into DMA pieces; keep the pieces for
    # chunks 0-1 fine (and never spanning a chunk boundary there) so the PE
    # can start early and never wait on a later chunk's data
    pieces = []   # (src, lo, hi)
    cur = None
    cur_c = -1
    for src, lo, hi, c in stream:
        limit = 1024 if c == 0 else (1600 if c < 3 else PDMA_COLS)
        if (cur is not None and cur[0] == src
                and (cur[2] - cur[1]) < limit
                and (c >= 2 or c == cur_c)):
            cur = (src, cur[1], hi)
        else:
            if cur is not None:
                pieces.append(cur)
            cur = (src, lo, hi)
            cur_c = c
    pieces.append(cur)

    # piece index that covers a given column of t/p
    def piece_covering(src, col):
        for k, (s_, lo, hi) in enumerate(pieces):
            if s_ == src and lo <= col < hi:
                return k
        raise AssertionError

    # per-chunk matmul rhs column offsets (aligned with T block order), and
    # the stream-piece index each matmul depends on; chunks 8-15 get their T
    # via DVE mirror copies (tracked by a per-chunk copy semaphore instead)
    mm = []   # entries: (sbuf t col, ppack col)
    mm_req = []
    for c in range(NCH):
        lst = []
        req = []
        for ci, ws in enumerate(classes):
            r = (CW * c) // ws
            for g in range(nb[(ci, r)]):
                pc = p_off[(ci, r)] + g * PAIR
                tc_ = tbase[(c, ci, g)]
                rq = 1 + piece_covering("p", pc + PAIR - 1)
                if c < NCH // 2:
                    rq = max(rq, 1 + piece_covering("t", tc_ + 127))
                lst.append((tc_, pc))
                req.append(rq)
        if c >= NCH // 2:
            r24 = (CW * c) // ws24
            lc = (CW * c - ws24 * r24) // CW
            pc = patch_p[r24]
            tq = patch_q[r24] + lc * 128
            lst.append((tco + tq, pc))
            req.append(1 + max(piece_covering("p", pc + PAIR - 1),
                               piece_covering("q", tq + 127)))
        mm.append(lst)
        mm_req.append(req)
    # DVE mirror-copy source requirement: pieces covering chunk 15-c's T
    copy_req = {}
    for c in range(NCH // 2, NCH):
        src = 15 - c
        copy_req[c] = 1 + piece_covering("t", t_off[src + 1] - 1)

    return {
        "T": T,
        "classes": classes,
        "chosen": chosen,
        "lo_rep": lo_rep,
        "items": items,
        "nb": nb,
        "p_off": p_off,
        "pieces": pieces,
        "tbase": tbase,
        "t_off": t_off,
        "mm": mm,
        "mm_req": mm_req,
        "copy_req": copy_req,
        "tcols": tco,
        "qcols": qcols,
        "tleft": t_off[NCH // 2],
        "tcols_dram": t_off[NCH // 2] + qcols,
        "pcols": pcols,
        "patch_p": patch_p,
        "patch_q": patch_q,
        "ws24": ws24,
        "wd24": wd24,
    }


def _build_inputs(image: np.ndarray, plan):
    """Per-core packed T / packed p (bf16).

    Pair x<=24 uses T_x directly (acc1 column = p_x, acc2 = p_{x+48});
    pair x>=25 uses the other 180-degree rep T_{x+48} == wflip(T_{48-x})
    (acc1 column = p_{x+48}, acc2 = p_x).  Only left-half chunks' T columns
    exist in DRAM; the right half is DVE-mirrored on the device.
    """
    T = plan["T"]
    classes = plan["classes"]
    items = plan["items"]
    p_off = plan["p_off"]
    tbase = plan["tbase"]
    tdram = plan["tcols_dram"]
    p = image.transpose(2, 1, 0, 3).reshape(U, NANG, BZ)
    in_maps = []
    for core in range(NCORES):
        hs = slice(HPC * core, HPC * (core + 1))
        tpack = np.zeros((128, tdram), dtype=ml_dtypes.bfloat16)
        ppack = np.zeros((128, plan["pcols"]), dtype=ml_dtypes.bfloat16)
        for ci, ws in enumerate(classes):
            R = W // ws
            nlc = ws // CW
            lo = plan["lo_rep"][ws]
            for r in range(R):
                c0 = (ws * r) // CW
                pc0 = p_off[(ci, r)]
                left = c0 < NCH // 2
                for a, gpos, w_ in items[(ci, r)]:
                    if left:
                        k0 = int(lo[a, core, r])
                    else:
                        # right-region rows are DVE mirror copies of the
                        # left twin's block, so the p rows must use the
                        # twin's band start (fp32 band jitter would
                        # misalign self-mirrored pairs otherwise)
                        k0 = int(lo[(NA2 - a) % NA2, core, R - 1 - r])
                    if k0 == 999:
                        continue
                    k0 = min(k0, 128 - w_)
                    if left:
                        if a < 25:
                            tb = T[a, k0:k0 + w_, hs, ws * r:ws * (r + 1)]
                        else:
                            # rep = wflip(T_{48-a}): left-region cols map to
                            # the mirrored right region, reversed
                            tb = T[NA2 - a, k0:k0 + w_, hs,
                                   W - ws * (r + 1):W - ws * r][:, :, ::-1]
                        blk = tb.reshape(w_, HPC, nlc, CW)
                    if not left and a == 24:
                        # wflip(T_24) == T_72 would duplicate contributions
                        # already covered from the left side; its p rows stay
                        # zero and the streamed patch blocks below supply the
                        # missing (24,right)/(72,left) combinations
                        continue
                    a1 = a if a < 25 else a + NA2      # acc1 angle index
                    a2 = a + NA2 if a < 25 else a      # acc2 angle index
                    i = 0
                    while i < w_:
                        row = (gpos + i) % 128
                        g = (gpos + i) // 128
                        n = min(w_ - i, 128 - row)
                        if left:
                            for lc in range(nlc):
                                c0t = tbase[(c0 + lc, ci, g)]
                                tpack[row:row + n, c0t:c0t + 128] = \
                                    blk[i:i + n, :, lc, :].reshape(n, 128)
                        pc = pc0 + g * PAIR
                        ppack[row:row + n, pc:pc + BZ] = \
                            p[k0 + i:k0 + i + n, a1, :]
                        ppack[row:row + n, pc + BZ:pc + PAIR] = \
                            p[k0 + i:k0 + i + n, a2, :]
                        i += n
        # pair-24 patch blocks: true T_24 at right-region columns + its
        # canonical p pair, at fresh rows 0..w
        ws24 = plan["ws24"]
        nlc24 = ws24 // CW
        lo24 = plan["lo_rep"][ws24]
        tleft = plan["tleft"]
        for r24, pc in plan["patch_p"].items():
            w_ = plan["wd24"][r24]
            k0 = int(lo24[24, core, r24])
            if k0 == 999 or w_ <= 0:
                continue
            k0 = min(k0, 128 - w_)
            tb = T[24, k0:k0 + w_, hs, ws24 * r24:ws24 * (r24 + 1)]
            blk = tb.reshape(w_, HPC, nlc24, CW)
            q0 = tleft + plan["patch_q"][r24]
            for lc in range(nlc24):
                tpack[:w_, q0 + lc * 128:q0 + (lc + 1) * 128] = \
                    blk[:, :, lc, :].reshape(w_, 128)
            ppack[:w_, pc:pc + BZ] = p[k0:k0 + w_, 24, :]
            ppack[:w_, pc + BZ:pc + PAIR] = p[k0:k0 + w_, 24 + NA2, :]
        in_maps.append({"tmat": np.ascontiguousarray(tpack),
                        "ppack": np.ascontiguousarray(ppack)})
    return in_maps


def _build_program_raw(plan):
    import concourse.bass as bass
    import concourse.mybir as mybir

    t_off = plan["t_off"]
    mm = plan["mm"]
    pieces = plan["pieces"]

    nc = bass.Bass(trn_type="TRN2")
    bf16 = mybir.dt.bfloat16
    f32 = mybir.dt.float32

    t_dram = nc.dram_tensor("tmat", [128, plan["tcols_dram"]], bf16,
                            kind="ExternalInput")
    p_dram = nc.dram_tensor("ppack", [128, plan["pcols"]], bf16,
                            kind="ExternalInput")
    o_dram = nc.dram_tensor("out", [128, NCH * PAIR], bf16,
                            kind="ExternalOutput")

    NPD = len(pieces)
    from contextlib import ExitStack
    with ExitStack() as stack:
        ec = stack.enter_context
        # T + p + out all fit in SBUF at once, so everything is resident:
        # no buffer recycling, and the SP load stream never has to wait.
        # Count-based waits on one semaphore are only safe when the DMAs
        # complete in issue order; hardware spreads DMAs over several rings,
        # so every load DMA gets its own semaphore.
        s_ps = [ec(nc.semaphore(f"s_p{i}")) for i in range(NPD)]
        s_tc = [ec(nc.semaphore(f"s_tc{i}")) for i in range(NCH // 2)]
        s_mm = ec(nc.semaphore("s_mm"))
        s_cp = ec(nc.semaphore("s_cp"))
        s_out = ec(nc.semaphore("s_out"))
        pp_sb = ec(nc.sbuf_tensor("pp_sb", [128, plan["pcols"]], bf16))
        t_all = ec(nc.sbuf_tensor("t_all", [128, plan["tcols"] + plan["qcols"]],
                                  bf16))
        o_all = ec(nc.sbuf_tensor("o_all", [128, NCH * PAIR], bf16))
        psums = [ec(nc.psum_tensor(f"ps{i}", [128, 512], f32))
                 for i in range(4)]
        ps_dummy = ec(nc.psum_tensor("ps_dummy", [128, 512], f32))
        banks = [ps_[:, :512] for ps_ in psums]

        with nc.Block() as block:

            @block.sync
            def _(sync):
                # SP: a pure load stream with NO waits (T/p fully resident),
                # pieces in PE consumption order, then the out stores
                tleft = plan["tleft"]
                tcols = plan["tcols"]
                for k, (src, lo_, hi_) in enumerate(pieces):
                    if src == "p":
                        sync.dma_start(
                            pp_sb[:, lo_:hi_], p_dram[:, lo_:hi_]
                        ).then_inc(s_ps[k], 16)
                    elif src == "t":
                        sync.dma_start(
                            t_all[:, lo_:hi_], t_dram[:, lo_:hi_]
                        ).then_inc(s_ps[k], 16)
                    else:
                        # "q": pair-24 patch, stored after the left T cols in
                        # DRAM and after all mirrored cols in SBUF
                        sync.dma_start(
                            t_all[:, tcols + lo_:tcols + hi_],
                            t_dram[:, tleft + lo_:tleft + hi_],
                        ).then_inc(s_ps[k], 16)
                store_at = [0, 8, 12, 14, 15, 16]
                for k in range(len(store_at) - 1):
                    sync.wait_ge(s_cp, store_at[k + 1])
                    sync.dma_start(
                        o_dram[:, store_at[k] * PAIR:store_at[k + 1] * PAIR],
                        o_all[:, store_at[k] * PAIR:store_at[k + 1] * PAIR],
                    ).then_inc(s_out, 16)
                sync.wait_ge(s_out, 16 * (len(store_at) - 1))

            @block.tensor
            def _(tensor):
                mm_req = plan["mm_req"]
                seen_p = 0
                for c in range(NCH):
                    nbk = len(mm[c])
                    if c >= NCH // 2:
                        tensor.wait_ge(s_tc[c - NCH // 2], 1)
                    if c >= 4:
                        tensor.wait_ge(s_cp, c - 3)
                    ps = banks[c % 4]
                    for i, (tc_, pc) in enumerate(mm[c]):
                        while seen_p < mm_req[c][i]:
                            tensor.wait_ge(s_ps[seen_p], 16)
                            seen_p += 1
                        mmi = tensor.matmul(
                            ps,
                            t_all[:, tc_:tc_ + 128],
                            pp_sb[:, pc:pc + PAIR],
                            start=(i == 0),
                            stop=(i == nbk - 1),
                        )
                        if i == nbk - 1:
                            mmi.then_inc(s_mm, 1)
                tensor.matmul(
                    ps_dummy[:, :BZ],
                    pp_sb[:, :128],
                    pp_sb[:, :BZ],
                    start=True,
                    stop=True,
                ).then_inc(s_mm, 1)

            @block.vector
            def _(vector):
                copy_req = plan["copy_req"]
                dve_seen = 0

                def tcopy(c):
                    nonlocal dve_seen
                    while dve_seen < copy_req[c]:
                        vector.wait_ge(s_ps[dve_seen], 16)
                        dve_seen += 1
                    src = 15 - c
                    ncols = t_off[c + 1] - t_off[c]
                    nblk = ncols // 128
                    dst = t_all[:, t_off[c]:t_off[c + 1]].rearrange(
                        "p (b hl wl) -> p b hl wl", hl=HPC, wl=CW)
                    s = t_all[:, t_off[src]:t_off[src + 1]].rearrange(
                        "p (b hl wl) -> p b hl wl", hl=HPC, wl=CW)[:, :, :, ::-1]
                    vector.tensor_copy(dst, s).then_inc(s_tc[c - NCH // 2], 1)

                def drain(c):
                    vector.wait_ge(s_mm, c + 2)
                    vector.tensor_copy(
                        o_all[:, c * PAIR:(c + 1) * PAIR], banks[c % 4]
                    ).then_inc(s_cp, 1)

                # mirror copies as their sources land (chunk 15's source is
                # the first T piece), interleaved so no drain is ever queued
                # behind a copy whose source hasn't arrived yet
                seqn = ["c15", "c14", "d0", "c13", "d1", "c12", "d2",
                        "c11", "d3", "c10", "c9", "c8"] + \
                       [f"d{j}" for j in range(4, NCH)]
                for op in seqn:
                    if op[0] == "c":
                        tcopy(int(op[1:]))
                    else:
                        drain(int(op[1:]))

    nc.finalize()
    return nc


def kernel(image: np.ndarray, angles: np.ndarray) -> np.ndarray:
    from concourse.bass_utils import run_bass_kernel_spmd

    image = np.ascontiguousarray(image, dtype=np.float32)
    angles = np.ascontiguousarray(angles, dtype=np.float32)

    key = angles.tobytes()
    if key not in _cache:
        plan = _make_plan(angles)
        nc = _build_program_raw(plan)
        _cache[key] = (nc, plan)
    nc, plan = _cache[key]

    in_maps = _build_inputs(image, plan)
    res = run_bass_kernel_spmd(nc, in_maps, core_ids=list(range(NCORES)))

    acc = np.empty((2, B, H, W, L), dtype=np.float32)
    for core in range(NCORES):
        o = np.asarray(res.results[core]["out"]).astype(np.float32)
        # cols = chunk(16) * [acc1 256 | acc2 256] ; px = hl*8+wl ; w = c*8+wl
        o = o.reshape(HPC, CW, NCH, 2, B, L)       # [hl, wl, c, acc, b, z]
        o = o.transpose(3, 4, 0, 2, 1, 5)          # [acc, b, hl, c, wl, z]
        acc[:, :, HPC * core:HPC * (core + 1)] = o.reshape(2, B, HPC, W, L)
    out = acc[0] + acc[1][:, ::-1, ::-1, :]
    return np.ascontiguousarray(out, dtype=np.float32)
